# revision 9
# baseline (speedup 1.0000x reference)
"""Trainium2 Bass kernel for KANPolyLayer:
    y[b,o] = sum_{i,p} x[b,i]^p * coeffs[o,i,p] + bias[o],  p = 0..4

Math: y = sum_{p=1..4} (x^p) @ C_p^T + (bias + colsum(C_0)), with
C_p = coeffs[:, :, p].  The p=0 plane and bias are folded on the host
(cheap O(out_dim*in_dim) reduction + broadcast add on gather); the
device does 4 accumulated GEMM planes in bf16 with powers computed
on-chip by the vector engine (x^2 = x*x, x^3 = x^2*x, x^4 = x^2*x^2).

Per-core schedule: everything is SBUF-resident (no tile rings).  The
k=0 slices of x and all four coefficient planes are host-packed into
one "head" DRAM tensor so the startup-critical data arrives as three
self-sufficient wide-row DMAs on the fast Sync HWDGE queue; the k>=1
bulk streams as large-row chunks (Sync: x, Scalar: k-major coeffs)
sized so every (k,p) plane lands ~2us+ before its matmuls (DMA packet
rate is ~fixed per packet, so bandwidth scales with row size).  A
warmup burst of garbage matmuls with no input dependencies keeps the
PE busy from the moment the engines start, so the HAM clock-gate
reaches 2.4 GHz right as the real stream begins.  All 8 (o-tile,
b-half) output groups accumulate concurrently in the 8 PSUM banks;
the trailing 2 k-planes are emitted group-contiguous so each group's
PSUM->SBUF copy (DVE) and output DMA overlap the remaining matmul
stream.  The kernel computes yT = [o, b]; the host transposes and
adds the folded bias row.

Sharding (8 cores): 4 batch groups x 2 out-dim groups.
  core c -> (bg, og) = (c // 2, c % 2)
Each core computes a disjoint (512 x 1024) block of yT; host gathers.
"""

from contextlib import ExitStack

import numpy as np
import ml_dtypes

import concourse.bacc as bacc
import concourse.bass as bass
import concourse.mybir as mybir
import concourse.tile as tile
from concourse.bass_utils import run_bass_kernel_spmd

F32 = mybir.dt.float32
BF16 = mybir.dt.bfloat16
NP_BF16 = ml_dtypes.bfloat16

B, I, O = 4096, 1024, 1024  # batch, in_dim, out_dim
BW, OW = 4, 2               # batch groups x out-dim groups (8 cores)
BS, OS = B // BW, O // OW   # per-core batch (1024) and out (512)
NK = I // 128               # contraction k-tiles (8)
NT = OS // 128              # o-tiles (4)
NH = BS // 512              # b-halves (2)
NTAIL = 2                   # trailing k-planes emitted group-contiguous
NWARM = 22                  # warmup matmuls (N=128, cold ~107ns each)

_CACHE: dict = {}


def _build():
    nc = bacc.Bacc("TRN2", target_bir_lowering=False, debug=False, num_devices=8)

    # head[ki, :]: x k0h0 | ct k0p1 | x k0h1 | ct k0p2 | ct k0p3 | ct k0p4
    hd_d = nc.dram_tensor("head", [128, 3072], BF16, kind="ExternalInput")
    # xb_d[ki, (k-1)*1024 + h*512 + b'] = x[bg*1024 + h*512 + b', k*128 + ki]
    xb_d = nc.dram_tensor("xb", [128, (NK - 1) * BS], BF16, kind="ExternalInput")
    # cb_d[ki, (k-1)*2048 + (p-1)*512 + o'] = coeffs[og*512 + o', k*128 + ki, p]
    cb_d = nc.dram_tensor("cb", [128, (NK - 1) * 4 * OS], BF16,
                          kind="ExternalInput")
    yt = nc.dram_tensor("yt", [OS, BS], F32, kind="ExternalOutput")  # [o, b]

    with tile.TileContext(nc) as tc, ExitStack() as ctx:
        cons = ctx.enter_context(tc.tile_pool(name="cons", bufs=1))
        xpool = ctx.enter_context(tc.tile_pool(name="x", bufs=1))
        cpool = ctx.enter_context(tc.tile_pool(name="coef", bufs=1))
        ppool = ctx.enter_context(tc.tile_pool(name="pow", bufs=1))
        opool = ctx.enter_context(tc.tile_pool(name="out", bufs=1))
        pspool = ctx.enter_context(
            tc.tile_pool(name="ps", bufs=8, space=bass.MemorySpace.PSUM)
        )

        # 8 concurrent accumulation groups: (o-tile, b-half) -> one PSUM bank
        ps = {}
        for ot in range(NT):
            for h in range(NH):
                ps[(ot, h)] = pspool.tile(
                    [128, 512], F32, tag="ps", name=f"ps_{ot}_{h}"
                )

        # PE warmup: garbage matmuls on a memset tile, issued with no input
        # dependencies, sized to span until the first head chunk lands so
        # the HAM activity window stays continuously busy and the clock
        # reaches 2.4 GHz right as the real stream begins.
        wz = cons.tile([128, 128], BF16)
        nc.vector.memset(wz[:], 0.0)
        for _ in range(NWARM):
            nc.tensor.matmul(
                ps[(0, 0)][:, 0:128], wz[:], wz[:], start=True, stop=True,
                skip_group_check=True,
            )

        # ---- input DMAs: two parallel HWDGE issue queues ----
        hd = cons.tile([128, 3072], BF16, tag="hd", name="hd")
        xm = xpool.tile([128, 3072], BF16, tag="xm", name="xm")   # k1-3 x
        xe = xpool.tile([128, 4096], BF16, tag="xe", name="xe")   # k4-7 x
        cb = cpool.tile([128, (NK - 1) * 4 * OS], BF16, tag="cb", name="cb")

        # Sync queue (starts ~0.9us before Scalar): the three head chunks,
        # each self-sufficient for the next stretch of matmuls, then bulk x.
        nc.sync.dma_start(hd[:, 0:1024], hd_d[:, 0:1024])      # x k0h0 + ct k0p1
        nc.sync.dma_start(hd[:, 1024:2048], hd_d[:, 1024:2048])  # x k0h1 + ct k0p2
        nc.sync.dma_start(hd[:, 2048:3072], hd_d[:, 2048:3072])  # ct k0p3,p4
        nc.sync.dma_start(xm[:], xb_d[:, 0:3072])
        nc.sync.dma_start(xe[:], xb_d[:, 3072:7168])
        # Scalar queue: k-major coefficient bulk.
        nc.scalar.dma_start(cb[:, 0:2048], cb_d[:, 0:2048])        # k1
        nc.scalar.dma_start(cb[:, 2048:6144], cb_d[:, 2048:6144])  # k2,k3
        nc.scalar.dma_start(cb[:, 6144:14336], cb_d[:, 6144:14336])  # k4-7

        def xs(k, h):
            """x^1 slice [128, 512] for k-tile k, half h."""
            if k == 0:
                return hd[:, 0:512] if h == 0 else hd[:, 1024:1536]
            t, base = (xm, 1) if k < 4 else (xe, 4)
            off = (k - base) * 1024 + h * 512
            return t[:, off:off + 512]

        def wslice(k, p, ot):
            if k == 0:
                base = (512, 1536, 2048, 2560)[p - 1] + ot * 128
                return hd[:, base:base + 128]
            base = (k - 1) * 2048 + (p - 1) * 512 + ot * 128
            return cb[:, base:base + 128]

        # ---- powers on DVE, per k-tile ----
        p2 = ppool.tile([128, NK * BS], BF16, tag="p2", name="p2")
        p3 = ppool.tile([128, NK * BS], BF16, tag="p3", name="p3")
        p4 = ppool.tile([128, NK * BS], BF16, tag="p4", name="p4")

        def pows(p, k, h):
            if p == 1:
                return xs(k, h)
            t = (None, None, p2, p3, p4)[p]
            off = k * 1024 + h * 512
            return t[:, off:off + 512]

        for h in range(NH):  # k0 halves live in separate head regions
            s, d0 = xs(0, h), h * 512
            nc.vector.tensor_mul(p2[:, d0:d0 + 512], s, s)
            nc.vector.tensor_mul(p3[:, d0:d0 + 512], p2[:, d0:d0 + 512], s)
            nc.vector.tensor_mul(p4[:, d0:d0 + 512], p2[:, d0:d0 + 512],
                                 p2[:, d0:d0 + 512])
        for k in range(1, NK):
            t, base = (xm, 1) if k < 4 else (xe, 4)
            s = t[:, (k - base) * 1024:(k - base) * 1024 + 1024]
            d0 = k * 1024
            nc.vector.tensor_mul(p2[:, d0:d0 + 1024], s, s)
            nc.vector.tensor_mul(p3[:, d0:d0 + 1024], p2[:, d0:d0 + 1024], s)
            nc.vector.tensor_mul(p4[:, d0:d0 + 1024], p2[:, d0:d0 + 1024],
                                 p2[:, d0:d0 + 1024])

        def mm(k, p, ot, h, start, stop):
            nc.tensor.matmul(ps[(ot, h)], wslice(k, p, ot), pows(p, k, h),
                             start=start, stop=stop)

        # main stream: k-major; h-outer so the first matmuls only need the
        # first head chunk (x k0h0 + ct k0p1)
        for k in range(NK - NTAIL):
            for p in range(1, 5):
                for h in range(NH):
                    for ot in range(NT):
                        mm(k, p, ot, h, start=(k == 0 and p == 1), stop=False)

        # trailing k-planes group-contiguous: groups finish ~1.8us apart so
        # each PSUM->SBUF copy + output DMA overlaps the matmul stream
        for gi, (ot, h) in enumerate([(ot, h) for ot in range(NT) for h in range(NH)]):
            for k in range(NK - NTAIL, NK):
                for p in range(1, 5):
                    mm(k, p, ot, h, start=False, stop=(k == NK - 1 and p == 4))
            o_sb = opool.tile([128, 512], F32, tag=f"o{gi}", name=f"o_{ot}_{h}")
            nc.vector.tensor_copy(o_sb[:], ps[(ot, h)][:])
            eng = nc.sync if gi % 2 == 0 else nc.scalar
            eng.dma_start(
                yt[ot * 128:(ot + 1) * 128, h * 512:(h + 1) * 512], o_sb[:]
            )

    nc.compile()
    return nc


def _get_nc():
    if "nc" not in _CACHE:
        _CACHE["nc"] = _build()
    return _CACHE["nc"]


def _pack_x(xs_block):
    # [1024b, 1024i] -> [ki, k*1024 + h*512 + b']
    a = xs_block.astype(NP_BF16)
    return np.ascontiguousarray(
        a.reshape(NH, 512, NK, 128).transpose(3, 2, 0, 1).reshape(128, NK * BS)
    )


def _pack_c(c_block):
    # [512o', 1024i, 4p] -> [ki, k*2048 + (p-1)*512 + o']
    a = c_block.astype(NP_BF16)
    return np.ascontiguousarray(
        a.transpose(1, 2, 0).reshape(NK, 128, 4, OS).transpose(1, 0, 2, 3)
        .reshape(128, NK * 4 * OS)
    )


def _make_in_maps(x, coeffs):
    x = np.asarray(x, dtype=np.float32)
    coeffs = np.asarray(coeffs, dtype=np.float32)
    xts = [_pack_x(x[bg * BS:(bg + 1) * BS, :]) for bg in range(BW)]
    cts = [_pack_c(coeffs[og * OS:(og + 1) * OS, :, 1:5]) for og in range(OW)]
    in_maps = []
    for c in range(BW * OW):
        bg, og = c // OW, c % OW
        xk, ck = xts[bg], cts[og]
        head = np.concatenate(
            [
                xk[:, 0:512],      # x k0 h0
                ck[:, 0:512],      # ct k0 p1
                xk[:, 512:1024],   # x k0 h1
                ck[:, 512:1024],   # ct k0 p2
                ck[:, 1024:2048],  # ct k0 p3, p4
            ],
            axis=1,
        )
        in_maps.append(
            {
                "head": np.ascontiguousarray(head),
                "xb": np.ascontiguousarray(xk[:, 1024:8192]),
                "cb": np.ascontiguousarray(ck[:, 2048:16384]),
            }
        )
    return in_maps


def _gather(results, base):
    y = np.empty((B, O), dtype=np.float32)
    for c, res in enumerate(results):
        bg, og = c // OW, c % OW
        y[bg * BS:(bg + 1) * BS, og * OS:(og + 1) * OS] = (
            res["yt"].T + base[og * OS:(og + 1) * OS]
        )
    return y


def run(x, coeffs, bias, trace=False, **trace_kwargs):
    nc = _get_nc()
    in_maps = _make_in_maps(x, coeffs)
    # p=0 plane (x^0 == 1) and bias folded on host:
    base = (
        np.asarray(coeffs, dtype=np.float32)[:, :, 0].sum(axis=1)
        + np.asarray(bias, dtype=np.float32)[0]
    )
    br = run_bass_kernel_spmd(
        nc, in_maps, list(range(BW * OW)), trace=trace, **trace_kwargs
    )
    return _gather(br.results, base), br


def kernel(x, coeffs, bias):
    out, _ = run(x, coeffs, bias)
    return out


# revision 10
# speedup vs baseline: 1.0780x; 1.0780x over previous
"""Trainium2 Bass kernel for KANPolyLayer:
    y[b,o] = sum_{i,p} x[b,i]^p * coeffs[o,i,p] + bias[o],  p = 0..4

Math: y = sum_{p=1..4} (x^p) @ C_p^T + (bias + colsum(C_0)), with
C_p = coeffs[:, :, p].  The p=0 plane and bias are folded on the host
(cheap O(out_dim*in_dim) reduction + broadcast add on gather); the
device does 4 accumulated GEMM planes in bf16 with powers computed
on-chip by the vector engine (x^2 = x*x, x^3 = x^2*x, x^4 = x^2*x^2).

Per-core schedule: everything is SBUF-resident (no tile rings).  DMA
packet rate is roughly fixed per packet (packet == row), so bandwidth
scales with row size, and completion semaphores fire up to ~2us after
the last packet; chunks are ordered across the two HWDGE issue queues
(Sync starts ~1us before Scalar) so every (k,p) plane lands with
multiple microseconds of slack before its matmuls even on a slow run:
a starved matmul stream does not just stall, it re-throttles the HAM
clock gate to 1.2 GHz.  Coefficients use a k-major DRAM layout
([ki, k*2048 + (p-1)*512 + o']) so bulk chunks have 4KB+ rows.  A
warmup burst of garbage matmuls with no input dependencies keeps the
PE busy from the moment the engines start, so the HAM clock reaches
2.4 GHz right as the real stream begins.  All 8 (o-tile, b-half)
output groups accumulate concurrently in the 8 PSUM banks; the
trailing 2 k-planes are emitted group-contiguous so each group's
PSUM->SBUF copy (DVE) and output DMA overlap the remaining matmul
stream.  The kernel computes yT = [o, b]; the host transposes and
adds the folded bias row.

Sharding (8 cores): 4 batch groups x 2 out-dim groups.
  core c -> (bg, og) = (c // 2, c % 2)
Each core computes a disjoint (512 x 1024) block of yT; host gathers.
"""

from contextlib import ExitStack

import numpy as np
import ml_dtypes

import concourse.bacc as bacc
import concourse.bass as bass
import concourse.mybir as mybir
import concourse.tile as tile
from concourse.bass_utils import run_bass_kernel_spmd

F32 = mybir.dt.float32
BF16 = mybir.dt.bfloat16
NP_BF16 = ml_dtypes.bfloat16

B, I, O = 4096, 1024, 1024  # batch, in_dim, out_dim
BW, OW = 4, 2               # batch groups x out-dim groups (8 cores)
BS, OS = B // BW, O // OW   # per-core batch (1024) and out (512)
NK = I // 128               # contraction k-tiles (8)
NT = OS // 128              # o-tiles (4)
NH = BS // 512              # b-halves (2)
NTAIL = 2                   # trailing k-planes emitted group-contiguous
NWARM = 30                  # warmup matmuls (N=128, cold ~107ns each)

_CACHE: dict = {}


def _build():
    nc = bacc.Bacc("TRN2", target_bir_lowering=False, debug=False, num_devices=8)

    # xt[ki, k*1024 + h*512 + b'] = x[bg*1024 + h*512 + b', k*128 + ki]
    xt = nc.dram_tensor("xt", [128, NK * BS], BF16, kind="ExternalInput")
    # ctk[ki, k*2048 + (p-1)*512 + o'] = coeffs[og*512 + o', k*128 + ki, p]
    ctk = nc.dram_tensor("ctk", [128, NK * 4 * OS], BF16, kind="ExternalInput")
    yt = nc.dram_tensor("yt", [OS, BS], F32, kind="ExternalOutput")  # [o, b]

    with tile.TileContext(nc) as tc, ExitStack() as ctx:
        cons = ctx.enter_context(tc.tile_pool(name="cons", bufs=1))
        xpool = ctx.enter_context(tc.tile_pool(name="x", bufs=1))
        cpool = ctx.enter_context(tc.tile_pool(name="coef", bufs=1))
        ppool = ctx.enter_context(tc.tile_pool(name="pow", bufs=1))
        opool = ctx.enter_context(tc.tile_pool(name="out", bufs=1))
        pspool = ctx.enter_context(
            tc.tile_pool(name="ps", bufs=8, space=bass.MemorySpace.PSUM)
        )

        # 8 concurrent accumulation groups: (o-tile, b-half) -> one PSUM bank
        ps = {}
        for ot in range(NT):
            for h in range(NH):
                ps[(ot, h)] = pspool.tile(
                    [128, 512], F32, tag="ps", name=f"ps_{ot}_{h}"
                )

        # PE warmup: garbage matmuls on a memset tile, issued with no input
        # dependencies, sized to span until the first input chunks land.
        wz = cons.tile([128, 128], BF16)
        nc.vector.memset(wz[:], 0.0)
        for _ in range(NWARM):
            nc.tensor.matmul(
                ps[(0, 0)][:, 0:128], wz[:], wz[:], start=True, stop=True,
                skip_group_check=True,
            )

        # ---- input DMAs: two parallel HWDGE issue queues ----
        xh = xpool.tile([128, 1024], BF16, tag="xh", name="xh")   # k0
        x1 = xpool.tile([128, 1024], BF16, tag="x1", name="x1")   # k1
        xm = xpool.tile([128, 2048], BF16, tag="xm", name="xm")   # k2,k3
        xe = xpool.tile([128, 4096], BF16, tag="xe", name="xe")   # k4-7
        ct = cpool.tile([128, NK * 4 * OS], BF16, tag="ct", name="ct")

        nc.sync.dma_start(xh[:], xt[:, 0:1024])                   # k0 x
        nc.scalar.dma_start(ct[:, 0:512], ctk[:, 0:512])          # k0 p1
        nc.sync.dma_start(ct[:, 512:1024], ctk[:, 512:1024])      # k0 p2
        nc.scalar.dma_start(ct[:, 1024:1536], ctk[:, 1024:1536])  # k0 p3
        nc.sync.dma_start(ct[:, 1536:2048], ctk[:, 1536:2048])    # k0 p4
        nc.scalar.dma_start(ct[:, 2048:4096], ctk[:, 2048:4096])  # k1 ct
        nc.sync.dma_start(x1[:], xt[:, 1024:2048])                # k1 x
        nc.scalar.dma_start(ct[:, 4096:8192], ctk[:, 4096:8192])  # k2,k3 ct
        nc.sync.dma_start(xm[:], xt[:, 2048:4096])                # k2,k3 x
        nc.scalar.dma_start(ct[:, 8192:16384], ctk[:, 8192:16384])  # k4-7 ct
        nc.sync.dma_start(xe[:], xt[:, 4096:8192])                # k4-7 x

        def xtile(k):
            if k == 0:
                return xh, 0
            if k == 1:
                return x1, 1
            return (xm, 2) if k < 4 else (xe, 4)

        def xs(k, h=None):
            """x^1 slice for k-tile k (both halves, or one half h)."""
            t, base = xtile(k)
            off = (k - base) * 1024 + (0 if h is None else h * 512)
            return t[:, off:off + (1024 if h is None else 512)]

        # ---- powers on DVE, per k-tile ----
        p2 = ppool.tile([128, NK * BS], BF16, tag="p2", name="p2")
        p3 = ppool.tile([128, NK * BS], BF16, tag="p3", name="p3")
        p4 = ppool.tile([128, NK * BS], BF16, tag="p4", name="p4")

        def pows(p, k, h):
            if p == 1:
                return xs(k, h)
            t = (None, None, p2, p3, p4)[p]
            off = k * 1024 + h * 512
            return t[:, off:off + 512]

        for k in range(NK):
            s = xs(k)
            d0 = k * 1024
            nc.vector.tensor_mul(p2[:, d0:d0 + 1024], s, s)
            nc.vector.tensor_mul(p3[:, d0:d0 + 1024], p2[:, d0:d0 + 1024], s)
            nc.vector.tensor_mul(p4[:, d0:d0 + 1024], p2[:, d0:d0 + 1024],
                                 p2[:, d0:d0 + 1024])

        def mm(k, p, ot, h, start, stop):
            base = k * 2048 + (p - 1) * 512 + ot * 128
            nc.tensor.matmul(ps[(ot, h)], ct[:, base:base + 128], pows(p, k, h),
                             start=start, stop=stop)

        # main stream: k-major, all 8 groups accumulate per (k, p) plane
        for k in range(NK - NTAIL):
            for p in range(1, 5):
                for h in range(NH):
                    for ot in range(NT):
                        mm(k, p, ot, h, start=(k == 0 and p == 1), stop=False)

        # trailing k-planes group-contiguous: groups finish ~1.8us apart so
        # each PSUM->SBUF copy + output DMA overlaps the matmul stream
        for gi, (ot, h) in enumerate([(ot, h) for ot in range(NT) for h in range(NH)]):
            for k in range(NK - NTAIL, NK):
                for p in range(1, 5):
                    mm(k, p, ot, h, start=False, stop=(k == NK - 1 and p == 4))
            o_sb = opool.tile([128, 512], F32, tag=f"o{gi}", name=f"o_{ot}_{h}")
            nc.vector.tensor_copy(o_sb[:], ps[(ot, h)][:])
            eng = nc.sync if gi % 2 == 0 else nc.scalar
            eng.dma_start(
                yt[ot * 128:(ot + 1) * 128, h * 512:(h + 1) * 512], o_sb[:]
            )

    nc.compile()
    return nc


def _get_nc():
    if "nc" not in _CACHE:
        _CACHE["nc"] = _build()
    return _CACHE["nc"]


def _pack_x(xs_block):
    # [1024b, 1024i] -> [ki, k*1024 + h*512 + b']
    a = xs_block.astype(NP_BF16)
    return np.ascontiguousarray(
        a.reshape(NH, 512, NK, 128).transpose(3, 2, 0, 1).reshape(128, NK * BS)
    )


def _pack_c(c_block):
    # [512o', 1024i, 4p] -> [ki, k*2048 + (p-1)*512 + o']
    a = c_block.astype(NP_BF16)
    return np.ascontiguousarray(
        a.transpose(1, 2, 0).reshape(NK, 128, 4, OS).transpose(1, 0, 2, 3)
        .reshape(128, NK * 4 * OS)
    )


def _make_in_maps(x, coeffs):
    x = np.asarray(x, dtype=np.float32)
    coeffs = np.asarray(coeffs, dtype=np.float32)
    xts = [_pack_x(x[bg * BS:(bg + 1) * BS, :]) for bg in range(BW)]
    cts = [_pack_c(coeffs[og * OS:(og + 1) * OS, :, 1:5]) for og in range(OW)]
    in_maps = []
    for c in range(BW * OW):
        bg, og = c // OW, c % OW
        in_maps.append({"xt": xts[bg], "ctk": cts[og]})
    return in_maps


def _gather(results, base):
    y = np.empty((B, O), dtype=np.float32)
    for c, res in enumerate(results):
        bg, og = c // OW, c % OW
        y[bg * BS:(bg + 1) * BS, og * OS:(og + 1) * OS] = (
            res["yt"].T + base[og * OS:(og + 1) * OS]
        )
    return y


def run(x, coeffs, bias, trace=False, **trace_kwargs):
    nc = _get_nc()
    in_maps = _make_in_maps(x, coeffs)
    # p=0 plane (x^0 == 1) and bias folded on host:
    base = (
        np.asarray(coeffs, dtype=np.float32)[:, :, 0].sum(axis=1)
        + np.asarray(bias, dtype=np.float32)[0]
    )
    br = run_bass_kernel_spmd(
        nc, in_maps, list(range(BW * OW)), trace=trace, **trace_kwargs
    )
    return _gather(br.results, base), br


def kernel(x, coeffs, bias):
    out, _ = run(x, coeffs, bias)
    return out
